# revision 10
# baseline (speedup 1.0000x reference)
"""Trainium2 Bass kernel for nn_BDHEncoder (BDH encoder, 6 tied layers).

Sharding: 8 cores = data-parallel over batch (2) x tensor-parallel over heads (4).
Each core computes its (batch, head) slice of every layer; the decoder GEMM
partial sums are AllReduced within each 4-core batch group; LayerNorm + residual
are computed replicated on every core of the group.

Device layout notes:
  - Neuron axis is permuted parity-major on the host (even originals first,
    odd second) so that RoPE pairs (2k, 2k+1) live at rows k and k+N/2 --
    tile i pairs with tile i+16 and shares one cos/sin (2048, T) table row k.
  - Activations x_sparse / qr / y_sparse are kept transposed: (N on partitions,
    T on free axis).  scores^T is built directly in (s, t) layout so it can be
    the stationary matmul operand of the ykv GEMM without a transpose.
  - Matmul operands are float16 (full PE rate, ~2^-11 rounding); PSUM
    accumulation, the x residual stream, RoPE tables/products and all LN
    statistics stay float32.
"""

import math
import os
import sys
from contextlib import ExitStack

import numpy as np

for _p in ("/opt/trn_rl_repo",):
    if os.path.isdir(_p) and _p not in sys.path:
        sys.path.insert(0, _p)

import concourse.bass as bass
import concourse.tile as tile
from concourse import mybir
from concourse.masks import make_identity, make_upper_triangular

# Problem constants (hardcoded per the self-contained-kernel contract).
B, T, D, NH, V = 2, 512, 256, 4, 256
N = 4096
NPAIR = N // 2
N_LAYER = 6
THETA = 2.0 ** 16
LN_EPS = 1e-5

P = 128
NT = N // P      # 32 neuron tiles
TP = T // P      # 4 t tiles
DT = D // P      # 2 d tiles
NPT = NPAIR // P # 16 pair tiles

F32 = mybir.dt.float32
F16 = mybir.dt.float16
NP_F16 = np.float16

N_CORES = 8
REPLICA_GROUPS = [[0, 1, 2, 3], [4, 5, 6, 7]]

# scores^T tile j covers free (t) range [SC_OFF[j], SC_OFF[j] + SC_W[j]).
SC_OFF = (0, 128, 256, 384)
SC_W = (512, 384, 256, 128)

Alu = mybir.AluOpType
Act = mybir.ActivationFunctionType


def _ln_stats(nc, stp, in_ap, eps_sb):
    """Return (mean, rstd) APs for LayerNorm along the free axis."""
    st = stp.tile([P, nc.vector.BN_STATS_DIM], F32, tag="bn", name="bn")
    nc.vector.bn_stats(out=st, in_=in_ap)
    mv = stp.tile([P, nc.vector.BN_AGGR_DIM], F32, tag="mv", name="mv")
    nc.vector.bn_aggr(out=mv, in_=st)
    rs = stp.tile([P, 1], F32, tag="rs", name="rs")
    nc.scalar.activation(out=rs, in_=mv[:, 1:2], func=Act.Sqrt, bias=eps_sb, scale=1.0)
    nc.vector.reciprocal(out=rs, in_=rs)
    return mv[:, 0:1], rs


def _ln_apply(nc, stp, out_ap, in_ap, eps_sb):
    mu, rs = _ln_stats(nc, stp, in_ap, eps_sb)
    nc.vector.tensor_scalar(
        out=out_ap, in0=in_ap, scalar1=mu, scalar2=rs,
        op0=Alu.subtract, op1=Alu.mult,
    )


def build_nc():
    nc = bass.Bass()

    # -------- I/O (fp16 for matmul operands, f32 elsewhere) --------
    x0_d = nc.declare_dram_parameter("x0", [T, D], F32, isOutput=False)
    xT0_d = nc.declare_dram_parameter("xT0", [D, T], F16, isOutput=False)
    x016_d = nc.declare_dram_parameter("x016", [T, D], F16, isOutput=False)
    enc_d = nc.declare_dram_parameter("enc", [D, N], F16, isOutput=False)
    encv_d = nc.declare_dram_parameter("encv", [D, N], F16, isOutput=False)
    dec_d = nc.declare_dram_parameter("dec", [N, D], F16, isOutput=False)
    cos_d = nc.declare_dram_parameter("cosT", [NPAIR, T], F16, isOutput=False)
    sin_d = nc.declare_dram_parameter("sinT", [NPAIR, T], F16, isOutput=False)
    lm_d = nc.declare_dram_parameter("lm_head", [D, V], F16, isOutput=False)

    logits_d = nc.declare_dram_parameter("logits", [T, V], F32, isOutput=True)
    emb_d = nc.declare_dram_parameter("emb", [1, D], F32, isOutput=True)
    trace_d = nc.declare_dram_parameter("trace", [P, NT], F32, isOutput=True)

    # Collective bounce buffers (collectives cannot touch I/O tensors).
    ar_in = nc.dram_tensor("ar_in", [T, D], F32)
    ar_out = nc.dram_tensor("ar_out", [T, D], F32)

    enc_r = enc_d.ap().rearrange("(dt p) n -> p dt n", p=P)
    encv_r = encv_d.ap().rearrange("(dt p) n -> p dt n", p=P)
    lm_r = lm_d.ap().rearrange("(dt p) v -> p dt v", p=P)
    x0_r = x0_d.ap().rearrange("(j p) d -> j p d", p=P)
    x016_r = x016_d.ap().rearrange("(j p) d -> j p d", p=P)
    xT0_r = xT0_d.ap().rearrange("(d p) t -> d p t", p=P)
    cos_r = cos_d.ap().rearrange("(i p) t -> i p t", p=P)
    sin_r = sin_d.ap().rearrange("(i p) t -> i p t", p=P)
    dec_r = dec_d.ap().rearrange("(k p) d -> k p d", p=P)
    ar_in_r = ar_in.ap().rearrange("(j p) d -> j p d", p=P)
    ar_out_r = ar_out.ap().rearrange("(j p) d -> j p d", p=P)
    logits_r = logits_d.ap().rearrange("(j p) v -> j p v", p=P)

    with ExitStack() as ctx:
        tc = ctx.enter_context(tile.TileContext(nc))
        constp = ctx.enter_context(tc.tile_pool(name="const", bufs=1))
        xsp = ctx.enter_context(tc.tile_pool(name="xs", bufs=NT))
        xyp = ctx.enter_context(tc.tile_pool(name="xy", bufs=NT))
        xp = ctx.enter_context(tc.tile_pool(name="x", bufs=2 * TP))
        x16p = ctx.enter_context(tc.tile_pool(name="x16", bufs=2 * TP))
        xTp = ctx.enter_context(tc.tile_pool(name="xT", bufs=2 * DT))
        spp = ctx.enter_context(tc.tile_pool(name="sp", bufs=TP))
        ykvp = ctx.enter_context(tc.tile_pool(name="ykv", bufs=2))
        ykvTp = ctx.enter_context(tc.tile_pool(name="ykvT", bufs=DT))
        wp = ctx.enter_context(tc.tile_pool(name="work", bufs=6))
        qp = ctx.enter_context(tc.tile_pool(name="q", bufs=3))
        csp = ctx.enter_context(tc.tile_pool(name="cs", bufs=2))
        ysp = ctx.enter_context(tc.tile_pool(name="ys", bufs=3))
        decp = ctx.enter_context(tc.tile_pool(name="decs", bufs=3))
        stp = ctx.enter_context(tc.tile_pool(name="stats", bufs=6))
        ps_acc = ctx.enter_context(tc.tile_pool(name="ps_acc", bufs=4, space="PSUM"))
        ps_mm = ctx.enter_context(tc.tile_pool(name="ps_mm", bufs=2, space="PSUM"))
        ps_trp = ctx.enter_context(tc.tile_pool(name="ps_tr", bufs=2, space="PSUM"))

        # -------- persistent constants --------
        enc_sb = constp.tile([P, DT, N], F16, tag="enc")
        nc.sync.dma_start(out=enc_sb, in_=enc_r)
        encv_sb = constp.tile([P, DT, N], F16, tag="encv")
        nc.sync.dma_start(out=encv_sb, in_=encv_r)
        lm_sb = constp.tile([P, DT, V], F16, tag="lm")
        nc.sync.dma_start(out=lm_sb, in_=lm_r)
        mask_sb = constp.tile([P, P], F32, tag="mask")
        make_upper_triangular(nc, mask_sb, 1.0, diag=False)  # 1 where s < t
        ident_sb = constp.tile([P, P], F16, tag="ident")
        make_identity(nc, ident_sb)
        eps_sb = constp.tile([P, 1], F32, tag="eps")
        nc.vector.memset(eps_sb, LN_EPS)
        ones_sb = constp.tile([P, 1], F16, tag="ones")
        nc.vector.memset(ones_sb, 1.0)
        trace_sb = constp.tile([P, NT], F32, tag="trace")

        x_tiles = []     # f32 residual stream
        x16_tiles = []   # fp16 shadow for matmul rhs
        for jt in range(TP):
            xt = xp.tile([P, D], F32, tag="x", name="x")
            nc.sync.dma_start(out=xt, in_=x0_r[jt])
            x_tiles.append(xt)
            xt16 = x16p.tile([P, D], F16, tag="x16", name="x16")
            nc.sync.dma_start(out=xt16, in_=x016_r[jt])
            x16_tiles.append(xt16)
        xT_tiles = []
        for d in range(DT):
            xtt = xTp.tile([P, T], F16, tag="xT", name="xT")
            nc.sync.dma_start(out=xtt, in_=xT0_r[d])
            xT_tiles.append(xtt)

        for layer in range(N_LAYER):
            last_layer = layer == N_LAYER - 1

            # ---- phase A: x_sparse (transposed) + RoPE + scores^T ----
            sc_ps = [ps_acc.tile([P, 512], F32, tag="acc", name="acc")[:, : SC_W[j]]
                     for j in range(TP)]
            xs_tiles = [None] * NT
            for i in range(NPT):
                for k in (i, NPT + i):
                    ps_x = ps_mm.tile([P, T], F32, tag="mm", name="mm")
                    for d in range(DT):
                        nc.tensor.matmul(
                            ps_x,
                            lhsT=enc_sb[:, d, k * P:(k + 1) * P],
                            rhs=xT_tiles[d],
                            start=(d == 0), stop=(d == DT - 1),
                        )
                    xs_k = xsp.tile([P, T], F16, tag="xs", name="xs")
                    nc.scalar.activation(out=xs_k, in_=ps_x, func=Act.Relu)
                    xs_tiles[k] = xs_k

                cos_i = csp.tile([P, T], F16, tag="cos", name="cos")
                nc.sync.dma_start(out=cos_i, in_=cos_r[i])
                sin_i = csp.tile([P, T], F16, tag="sin", name="sin")
                nc.sync.dma_start(out=sin_i, in_=sin_r[i])

                xe, xo = xs_tiles[i], xs_tiles[NPT + i]
                t1 = wp.tile([P, T], F16, tag="w", name="w")
                nc.vector.tensor_tensor(out=t1, in0=xe, in1=cos_i, op=Alu.mult)
                t2 = wp.tile([P, T], F16, tag="w", name="w")
                nc.vector.tensor_tensor(out=t2, in0=xo, in1=sin_i, op=Alu.mult)
                qe = qp.tile([P, T], F16, tag="q", name="q")
                nc.vector.tensor_sub(qe, t1, t2)
                t3 = wp.tile([P, T], F16, tag="w", name="w")
                nc.vector.tensor_tensor(out=t3, in0=xo, in1=cos_i, op=Alu.mult)
                t4 = wp.tile([P, T], F16, tag="w", name="w")
                nc.vector.tensor_tensor(out=t4, in0=xe, in1=sin_i, op=Alu.mult)
                qo = qp.tile([P, T], F16, tag="q", name="q")
                nc.vector.tensor_add(qo, t3, t4)

                for qi, q in enumerate((qe, qo)):
                    first = (i == 0) and (qi == 0)
                    last = (i == NPT - 1) and (qi == 1)
                    for j in range(TP):
                        nc.tensor.matmul(
                            sc_ps[j],
                            lhsT=q[:, j * P:(j + 1) * P],
                            rhs=q[:, SC_OFF[j]:SC_OFF[j] + SC_W[j]],
                            start=first, stop=last,
                        )

            # ---- phase B: mask scores, ykv = LN(scores @ x), transpose ----
            sp_tiles = []
            for j in range(TP):
                spt = spp.tile([P, T], F16, tag="sp", name="sp")
                # diagonal block: strictly-causal mask (s < t)
                nc.vector.tensor_tensor(
                    out=spt[:, j * P:(j + 1) * P],
                    in0=sc_ps[j][:, j * P - SC_OFF[j]:(j + 1) * P - SC_OFF[j]],
                    in1=mask_sb, op=Alu.mult,
                )
                if (j + 1) * P < T:
                    nc.scalar.activation(
                        out=spt[:, (j + 1) * P:],
                        in_=sc_ps[j][:, (j + 1) * P - SC_OFF[j]:],
                        func=Act.Copy,
                    )
                sp_tiles.append(spt)

            ykvT_tiles = [ykvTp.tile([P, T], F16, tag="ykvT", name="ykvT")
                          for _ in range(DT)]
            for jt in range(TP):
                ps_ykv = ps_mm.tile([P, D], F32, tag="mm", name="mm")
                for js in range(jt + 1):
                    nc.tensor.matmul(
                        ps_ykv,
                        lhsT=sp_tiles[js][:, jt * P:(jt + 1) * P],
                        rhs=x16_tiles[js],
                        start=(js == 0), stop=(js == jt),
                    )
                ykv_t = ykvp.tile([P, D], F16, tag="ykv", name="ykv")
                _ln_apply(nc, stp, ykv_t, ps_ykv, eps_sb)
                for d in range(DT):
                    ps_tr = ps_trp.tile([P, P], F16, tag="mmt", name="mmt")
                    nc.tensor.transpose(ps_tr, ykv_t[:, d * P:(d + 1) * P], ident_sb)
                    nc.vector.tensor_copy(
                        out=ykvT_tiles[d][:, jt * P:(jt + 1) * P], in_=ps_tr)

            # ---- phase C: y_sparse, gate, decoder GEMM partial ----
            ymlp_ps = [ps_acc.tile([P, D], F32, tag="acc", name="acc")
                       for _ in range(TP)]
            for k in range(NT):
                dec_k = decp.tile([P, D], F16, tag="dec", name="dec")
                nc.sync.dma_start(out=dec_k, in_=dec_r[k])
                ps_y = ps_mm.tile([P, T], F32, tag="mm", name="mm")
                for d in range(DT):
                    nc.tensor.matmul(
                        ps_y,
                        lhsT=encv_sb[:, d, k * P:(k + 1) * P],
                        rhs=ykvT_tiles[d],
                        start=(d == 0), stop=(d == DT - 1),
                    )
                # xy = relu(ps_y) * xs_k  (fp16 for the decoder GEMM)
                ys_k = ysp.tile([P, T], F16, tag="ys", name="ys")
                nc.scalar.activation(out=ys_k, in_=ps_y, func=Act.Relu)
                xy_k = xyp.tile([P, T], F16, tag="xy", name="xy")
                nc.vector.tensor_tensor(out=xy_k, in0=ys_k, in1=xs_tiles[k],
                                        op=Alu.mult)
                if last_layer:
                    nc.vector.reduce_sum(out=trace_sb[:, k:k + 1], in_=xy_k,
                                         axis=mybir.AxisListType.X)
                for jt in range(TP):
                    nc.tensor.matmul(
                        ymlp_ps[jt],
                        lhsT=xy_k[:, jt * P:(jt + 1) * P],
                        rhs=dec_k,
                        start=(k == 0), stop=(k == NT - 1),
                    )

            # ---- phase D: AllReduce ymlp over the 4-core head group ----
            for jt in range(TP):
                ym_sb = wp.tile([P, D], F32, tag="w", name="w")
                nc.scalar.activation(out=ym_sb, in_=ymlp_ps[jt], func=Act.Copy)
                nc.sync.dma_start(out=ar_in_r[jt], in_=ym_sb)
            nc.gpsimd.collective_compute(
                "AllReduce", Alu.add,
                replica_groups=REPLICA_GROUPS,
                ins=[ar_in.ap()],
                outs=[ar_out.ap()],
            )

            # x = LN(x + LN(ymlp)); also refresh fp16 shadow + x^T
            new_x, new_x16 = [], []
            for jt in range(TP):
                ym_t = wp.tile([P, D], F32, tag="w", name="w")
                nc.sync.dma_start(out=ym_t, in_=ar_out_r[jt])
                ln1 = wp.tile([P, D], F32, tag="w", name="w")
                _ln_apply(nc, stp, ln1, ym_t, eps_sb)
                nc.vector.tensor_add(ln1, ln1, x_tiles[jt])
                x_new = xp.tile([P, D], F32, tag="x", name="x")
                mu, rs = _ln_stats(nc, stp, ln1, eps_sb)
                nc.vector.tensor_scalar(
                    out=x_new, in0=ln1, scalar1=mu, scalar2=rs,
                    op0=Alu.subtract, op1=Alu.mult)
                x16_new = x16p.tile([P, D], F16, tag="x16", name="x16")
                nc.vector.tensor_scalar(
                    out=x16_new, in0=ln1, scalar1=mu, scalar2=rs,
                    op0=Alu.subtract, op1=Alu.mult)
                new_x.append(x_new)
                new_x16.append(x16_new)
            x_tiles, x16_tiles = new_x, new_x16
            new_xT = [xTp.tile([P, T], F16, tag="xT", name="xT")
                      for _ in range(DT)]
            for d in range(DT):
                for jt in range(TP):
                    ps_tr = ps_trp.tile([P, P], F16, tag="mmt", name="mmt")
                    nc.tensor.transpose(
                        ps_tr, x16_tiles[jt][:, d * P:(d + 1) * P], ident_sb)
                    nc.vector.tensor_copy(
                        out=new_xT[d][:, jt * P:(jt + 1) * P], in_=ps_tr)
            xT_tiles = new_xT

        # -------- epilogue: logits, embedding, neuron trace --------
        for jt in range(TP):
            ps_lg = ps_mm.tile([P, V], F32, tag="mm", name="mm")
            for d in range(DT):
                nc.tensor.matmul(
                    ps_lg,
                    lhsT=xT_tiles[d][:, jt * P:(jt + 1) * P],
                    rhs=lm_sb[:, d, :],
                    start=(d == 0), stop=(d == DT - 1),
                )
            lg_sb = wp.tile([P, V], F32, tag="w", name="w")
            nc.scalar.activation(out=lg_sb, in_=ps_lg, func=Act.Copy)
            nc.sync.dma_start(out=logits_r[jt], in_=lg_sb)

        ps_e = ps_mm.tile([1, D], F32, tag="mm", name="emb")
        for jt in range(TP):
            nc.tensor.matmul(
                ps_e, lhsT=ones_sb, rhs=x16_tiles[jt],
                start=(jt == 0), stop=(jt == TP - 1),
            )
        e_sb = wp.tile([1, D], F32, tag="embs", name="embs")
        nc.vector.tensor_scalar_mul(e_sb, ps_e, 1.0 / T)
        nc.sync.dma_start(out=emb_d.ap(), in_=e_sb)

        tr_sb = wp.tile([P, NT], F32, tag="trs", name="trs")
        nc.vector.tensor_scalar_mul(tr_sb, trace_sb, 1.0 / T)
        nc.sync.dma_start(out=trace_d.ap(), in_=tr_sb)

    # TRN2 allows only one sync-wait per compute instruction; these are the
    # Bacc-pipeline passes that move matmul waits onto ldweights and split
    # the remaining excess onto EventSemaphore instructions.  The
    # run_bass_kernel_spmd/axon compile path does not run them itself.
    import bass_rust
    bass_rust.move_matmul_waits_to_ldweights(nc.m)
    bass_rust.generate_event_semaphores(nc)

    return nc


# ---------------------------------------------------------------------------
# Host side
# ---------------------------------------------------------------------------

def _rope_tables():
    """(NPAIR, T) cos/sin tables, matching the reference's f32 arithmetic."""
    qidx = np.float32(2.0) * np.floor(
        np.arange(N, dtype=np.float32) / np.float32(2.0))
    freqs = (np.float32(1.0)
             / np.power(np.float32(THETA), qidx / np.float32(N))
             / np.float32(2.0 * math.pi)).astype(np.float32)
    phases = (np.arange(T, dtype=np.float32)[:, None] * freqs[None, :])
    ph = np.mod(phases, np.float32(1.0)) * np.float32(2.0 * math.pi)
    cos_full = np.cos(ph).astype(np.float32)   # (T, N)
    sin_full = np.sin(ph).astype(np.float32)
    cosP = np.ascontiguousarray(cos_full[:, 0::2].T)   # (NPAIR, T)
    sinP = np.ascontiguousarray(sin_full[:, 0::2].T)
    return cosP, sinP


def prepare_in_maps(input_ids, embed_table, encoder, encoder_v, decoder, lm_head):
    input_ids = np.asarray(input_ids).astype(np.int64)
    embed_table = np.asarray(embed_table, dtype=np.float32)
    encoder = np.asarray(encoder, dtype=np.float32)
    encoder_v = np.asarray(encoder_v, dtype=np.float32)
    decoder = np.asarray(decoder, dtype=np.float32)
    lm_head = np.asarray(lm_head, dtype=np.float32)

    # initial x = LN(embed[ids])
    x = embed_table[input_ids]                      # (B, T, D) f32
    mu = x.mean(-1, keepdims=True, dtype=np.float32)
    var = ((x - mu) ** 2).mean(-1, keepdims=True, dtype=np.float32)
    x0 = ((x - mu) / np.sqrt(var + np.float32(LN_EPS))).astype(np.float32)

    # parity-major neuron permutation: n' = (n % 2) * NPAIR + n // 2
    perm = np.concatenate([np.arange(0, N, 2), np.arange(1, N, 2)])
    enc_p = encoder[:, :, perm]                     # (NH, D, N)
    encv_p = encoder_v[:, :, perm]
    dec_p = decoder.reshape(NH, N, D)[:, perm, :]   # head-major like reference

    cosP, sinP = _rope_tables()
    cosP = cosP.astype(NP_F16)
    sinP = sinP.astype(NP_F16)
    lm16 = np.ascontiguousarray(lm_head.astype(NP_F16))

    in_maps = []
    for c in range(N_CORES):
        b, h = divmod(c, NH)
        in_maps.append({
            "x0": np.ascontiguousarray(x0[b]),
            "x016": np.ascontiguousarray(x0[b].astype(NP_F16)),
            "xT0": np.ascontiguousarray(x0[b].T.astype(NP_F16)),
            "enc": np.ascontiguousarray(enc_p[h].astype(NP_F16)),
            "encv": np.ascontiguousarray(encv_p[h].astype(NP_F16)),
            "dec": np.ascontiguousarray(dec_p[h].astype(NP_F16)),
            "cosT": cosP,
            "sinT": sinP,
            "lm_head": lm16,
        })
    return in_maps


def assemble_outputs(results):
    """results: list of 8 dicts with 'logits', 'emb', 'trace'."""
    logits = np.stack([np.asarray(results[b * NH]["logits"], dtype=np.float32)
                       for b in range(B)])                     # (B, T, V)
    embedding = np.stack([np.asarray(results[b * NH]["emb"], dtype=np.float32)[0]
                          for b in range(B)])                  # (B, D)
    # device trace[p, k] = mean_t xy at permuted neuron n' = k*128 + p
    nprime = (np.arange(N) % 2) * NPAIR + np.arange(N) // 2
    neuron_trace = np.zeros((B, NH * N), dtype=np.float32)
    for c in range(N_CORES):
        b, h = divmod(c, NH)
        t_dev = np.asarray(results[c]["trace"], dtype=np.float32)  # (P, NT)
        flat = t_dev.T.reshape(-1)                                 # index n'
        neuron_trace[b, h * N:(h + 1) * N] = flat[nprime]
    return logits, embedding, neuron_trace


_NC_CACHE = {}


def _get_nc():
    if "nc" not in _NC_CACHE:
        _NC_CACHE["nc"] = build_nc()
    return _NC_CACHE["nc"]


def run_on_hw(in_maps, trace=False):
    from concourse.bass_utils import run_bass_kernel_spmd
    nc = _get_nc()
    return run_bass_kernel_spmd(nc, in_maps, list(range(N_CORES)), trace=trace)


def kernel(input_ids, embed_table, encoder, encoder_v, decoder, lm_head):
    in_maps = prepare_in_maps(
        input_ids, embed_table, encoder, encoder_v, decoder, lm_head)
    res = run_on_hw(in_maps)
    return assemble_outputs(res.results)


# revision 12
# speedup vs baseline: 1.6732x; 1.6732x over previous
"""Trainium2 Bass kernel for nn_BDHEncoder (BDH encoder, 6 tied layers).

Sharding: 8 cores = data-parallel over batch (2) x tensor-parallel over heads (4).
Each core computes its (batch, head) slice of every layer; the decoder GEMM
partial sums are AllReduced within each 4-core batch group; LayerNorm + residual
are computed replicated on every core of the group.

Device layout notes:
  - Neuron axis is permuted parity-major on the host (even originals first,
    odd second) so that RoPE pairs (2k, 2k+1) live at rows k and k+N/2 --
    tile i pairs with tile i+16 and shares one cos/sin (2048, T) table row k.
  - Activations x_sparse / qr / y_sparse are kept transposed: (N on partitions,
    T on free axis).  scores^T is built directly in (s, t) layout so it can be
    the stationary matmul operand of the ykv GEMM without a transpose.
  - Matmul operands are float16 (full PE rate, ~2^-11 rounding); PSUM
    accumulation, the x residual stream, RoPE tables/products and all LN
    statistics stay float32.
"""

import math
import os
import sys
from contextlib import ExitStack

import numpy as np

for _p in ("/opt/trn_rl_repo",):
    if os.path.isdir(_p) and _p not in sys.path:
        sys.path.insert(0, _p)

import concourse.bass as bass
import concourse.tile as tile
from concourse import mybir
from concourse.masks import make_identity, make_upper_triangular

# Problem constants (hardcoded per the self-contained-kernel contract).
B, T, D, NH, V = 2, 512, 256, 4, 256
N = 4096
NPAIR = N // 2
N_LAYER = 6
THETA = 2.0 ** 16
LN_EPS = 1e-5

P = 128
NT = N // P      # 32 neuron tiles
TP = T // P      # 4 t tiles
DT = D // P      # 2 d tiles
NPT = NPAIR // P # 16 pair tiles

F32 = mybir.dt.float32
F16 = mybir.dt.float16
NP_F16 = np.float16

N_CORES = 8
REPLICA_GROUPS = [[0, 1, 2, 3], [4, 5, 6, 7]]

# scores^T tile j covers free (t) range [SC_OFF[j], SC_OFF[j] + SC_W[j]).
SC_OFF = (0, 128, 256, 384)
SC_W = (512, 384, 256, 128)

Alu = mybir.AluOpType
Act = mybir.ActivationFunctionType


def _ln_stats(nc, stp, in_ap, eps_sb):
    """Return (mean, rstd) APs for LayerNorm along the free axis."""
    st = stp.tile([P, nc.vector.BN_STATS_DIM], F32, tag="bn", name="bn")
    nc.vector.bn_stats(out=st, in_=in_ap)
    mv = stp.tile([P, nc.vector.BN_AGGR_DIM], F32, tag="mv", name="mv")
    nc.vector.bn_aggr(out=mv, in_=st)
    rs = stp.tile([P, 1], F32, tag="rs", name="rs")
    nc.scalar.activation(out=rs, in_=mv[:, 1:2], func=Act.Sqrt, bias=eps_sb, scale=1.0)
    nc.vector.reciprocal(out=rs, in_=rs)
    return mv[:, 0:1], rs


def _ln_apply(nc, stp, out_ap, in_ap, eps_sb):
    mu, rs = _ln_stats(nc, stp, in_ap, eps_sb)
    nc.vector.tensor_scalar(
        out=out_ap, in0=in_ap, scalar1=mu, scalar2=rs,
        op0=Alu.subtract, op1=Alu.mult,
    )


def build_nc():
    nc = bass.Bass()

    # -------- I/O (fp16 for matmul operands, f32 elsewhere) --------
    x0_d = nc.declare_dram_parameter("x0", [T, D], F32, isOutput=False)
    xT0_d = nc.declare_dram_parameter("xT0", [D, T], F16, isOutput=False)
    x016_d = nc.declare_dram_parameter("x016", [T, D], F16, isOutput=False)
    enc_d = nc.declare_dram_parameter("enc", [D, N], F16, isOutput=False)
    encv_d = nc.declare_dram_parameter("encv", [D, N], F16, isOutput=False)
    dec_d = nc.declare_dram_parameter("dec", [N, D], F16, isOutput=False)
    cos_d = nc.declare_dram_parameter("cosT", [NPAIR, T], F16, isOutput=False)
    sin_d = nc.declare_dram_parameter("sinT", [NPAIR, T], F16, isOutput=False)
    lm_d = nc.declare_dram_parameter("lm_head", [D, V], F16, isOutput=False)

    logits_d = nc.declare_dram_parameter("logits", [T, V], F32, isOutput=True)
    emb_d = nc.declare_dram_parameter("emb", [1, D], F32, isOutput=True)
    trace_d = nc.declare_dram_parameter("trace", [P, NT], F32, isOutput=True)

    # Collective bounce buffers (collectives cannot touch I/O tensors).
    ar_in = nc.dram_tensor("ar_in", [T, D], F16)
    ar_out = nc.dram_tensor("ar_out", [T, D], F16)

    enc_r = enc_d.ap().rearrange("(dt p) n -> p dt n", p=P)
    encv_r = encv_d.ap().rearrange("(dt p) n -> p dt n", p=P)
    lm_r = lm_d.ap().rearrange("(dt p) v -> p dt v", p=P)
    x0_r = x0_d.ap().rearrange("(j p) d -> j p d", p=P)
    x016_r = x016_d.ap().rearrange("(j p) d -> j p d", p=P)
    xT0_r = xT0_d.ap().rearrange("(d p) t -> d p t", p=P)
    cos_r = cos_d.ap().rearrange("(i p) t -> i p t", p=P)
    sin_r = sin_d.ap().rearrange("(i p) t -> i p t", p=P)
    dec_r = dec_d.ap().rearrange("(k p) d -> k p d", p=P)
    ar_in_r = ar_in.ap().rearrange("(j p) d -> j p d", p=P)
    ar_out_r = ar_out.ap().rearrange("(j p) d -> j p d", p=P)
    logits_r = logits_d.ap().rearrange("(j p) v -> j p v", p=P)

    with ExitStack() as ctx:
        tc = ctx.enter_context(tile.TileContext(nc))
        constp = ctx.enter_context(tc.tile_pool(name="const", bufs=1))
        xsp = ctx.enter_context(tc.tile_pool(name="xs", bufs=NT))
        xyp = ctx.enter_context(tc.tile_pool(name="xy", bufs=NT))
        xp = ctx.enter_context(tc.tile_pool(name="x", bufs=2 * TP))
        x16p = ctx.enter_context(tc.tile_pool(name="x16", bufs=2 * TP))
        xTp = ctx.enter_context(tc.tile_pool(name="xT", bufs=2 * DT))
        spp = ctx.enter_context(tc.tile_pool(name="sp", bufs=TP))
        ykvp = ctx.enter_context(tc.tile_pool(name="ykv", bufs=2))
        ykvTp = ctx.enter_context(tc.tile_pool(name="ykvT", bufs=DT))
        wp = ctx.enter_context(tc.tile_pool(name="work", bufs=8))
        qp = ctx.enter_context(tc.tile_pool(name="q", bufs=4))
        csp = ctx.enter_context(tc.tile_pool(name="cs", bufs=3))
        ysp = ctx.enter_context(tc.tile_pool(name="ys", bufs=4))
        decp = ctx.enter_context(tc.tile_pool(name="decs", bufs=4))
        stp = ctx.enter_context(tc.tile_pool(name="stats", bufs=6))
        ps_acc = ctx.enter_context(tc.tile_pool(name="ps_acc", bufs=4, space="PSUM"))
        ps_mm = ctx.enter_context(tc.tile_pool(name="ps_mm", bufs=3, space="PSUM"))

        # -------- persistent constants --------
        enc_sb = constp.tile([P, DT, N], F16, tag="enc")
        nc.sync.dma_start(out=enc_sb, in_=enc_r)
        encv_sb = constp.tile([P, DT, N], F16, tag="encv")
        nc.sync.dma_start(out=encv_sb, in_=encv_r)
        lm_sb = constp.tile([P, DT, V], F16, tag="lm")
        nc.sync.dma_start(out=lm_sb, in_=lm_r)
        mask_sb = constp.tile([P, P], F32, tag="mask")
        make_upper_triangular(nc, mask_sb, 1.0, diag=False)  # 1 where s < t
        ident_sb = constp.tile([P, P], F16, tag="ident")
        make_identity(nc, ident_sb)
        eps_sb = constp.tile([P, 1], F32, tag="eps")
        nc.vector.memset(eps_sb, LN_EPS)
        ones_sb = constp.tile([P, 1], F16, tag="ones")
        nc.vector.memset(ones_sb, 1.0)
        trace_sb = constp.tile([P, NT], F32, tag="trace")

        x_tiles = []     # f32 residual stream
        x16_tiles = []   # fp16 shadow for matmul rhs
        for jt in range(TP):
            xt = xp.tile([P, D], F32, tag="x", name="x")
            nc.sync.dma_start(out=xt, in_=x0_r[jt])
            x_tiles.append(xt)
            xt16 = x16p.tile([P, D], F16, tag="x16", name="x16")
            nc.sync.dma_start(out=xt16, in_=x016_r[jt])
            x16_tiles.append(xt16)
        xT_tiles = []
        for d in range(DT):
            xtt = xTp.tile([P, T], F16, tag="xT", name="xT")
            nc.sync.dma_start(out=xtt, in_=xT0_r[d])
            xT_tiles.append(xtt)

        for layer in range(N_LAYER):
            last_layer = layer == N_LAYER - 1

            # ---- phase A: x_sparse (transposed) + RoPE + scores^T ----
            sc_ps = [ps_acc.tile([P, 512], F32, tag="acc", name="acc")[:, : SC_W[j]]
                     for j in range(TP)]
            xs_tiles = [None] * NT
            for i in range(NPT):
                for k in (i, NPT + i):
                    ps_x = ps_mm.tile([P, T], F32, tag="mm", name="mm")
                    for d in range(DT):
                        nc.tensor.matmul(
                            ps_x,
                            lhsT=enc_sb[:, d, k * P:(k + 1) * P],
                            rhs=xT_tiles[d],
                            start=(d == 0), stop=(d == DT - 1),
                        )
                    xs_k = xsp.tile([P, T], F16, tag="xs", name="xs")
                    nc.scalar.activation(out=xs_k, in_=ps_x, func=Act.Relu)
                    xs_tiles[k] = xs_k

                cos_i = csp.tile([P, T], F16, tag="cos", name="cos")
                nc.sync.dma_start(out=cos_i, in_=cos_r[i])
                sin_i = csp.tile([P, T], F16, tag="sin", name="sin")
                nc.sync.dma_start(out=sin_i, in_=sin_r[i])

                xe, xo = xs_tiles[i], xs_tiles[NPT + i]
                t1 = wp.tile([P, T], F16, tag="w", name="w")
                nc.vector.tensor_tensor(out=t1, in0=xe, in1=cos_i, op=Alu.mult)
                t2 = wp.tile([P, T], F16, tag="w", name="w")
                nc.vector.tensor_tensor(out=t2, in0=xo, in1=sin_i, op=Alu.mult)
                qe = qp.tile([P, T], F16, tag="q", name="q")
                nc.vector.tensor_sub(qe, t1, t2)
                t3 = wp.tile([P, T], F16, tag="w", name="w")
                nc.vector.tensor_tensor(out=t3, in0=xo, in1=cos_i, op=Alu.mult)
                t4 = wp.tile([P, T], F16, tag="w", name="w")
                nc.vector.tensor_tensor(out=t4, in0=xe, in1=sin_i, op=Alu.mult)
                qo = qp.tile([P, T], F16, tag="q", name="q")
                nc.vector.tensor_add(qo, t3, t4)

                for qi, q in enumerate((qe, qo)):
                    first = (i == 0) and (qi == 0)
                    last = (i == NPT - 1) and (qi == 1)
                    for j in range(TP):
                        nc.tensor.matmul(
                            sc_ps[j],
                            lhsT=q[:, j * P:(j + 1) * P],
                            rhs=q[:, SC_OFF[j]:SC_OFF[j] + SC_W[j]],
                            start=first, stop=last,
                        )

            # ---- phase B: mask scores, ykv = LN(scores @ x), transpose ----
            sp_tiles = []
            for j in range(TP):
                spt = spp.tile([P, T], F16, tag="sp", name="sp")
                # diagonal block: strictly-causal mask (s < t)
                nc.vector.tensor_tensor(
                    out=spt[:, j * P:(j + 1) * P],
                    in0=sc_ps[j][:, j * P - SC_OFF[j]:(j + 1) * P - SC_OFF[j]],
                    in1=mask_sb, op=Alu.mult,
                )
                if (j + 1) * P < T:
                    nc.scalar.activation(
                        out=spt[:, (j + 1) * P:],
                        in_=sc_ps[j][:, (j + 1) * P - SC_OFF[j]:],
                        func=Act.Copy,
                    )
                sp_tiles.append(spt)

            ykvT_tiles = [ykvTp.tile([P, T], F16, tag="ykvT", name="ykvT")
                          for _ in range(DT)]
            for jt in range(TP):
                ps_ykv = ps_mm.tile([P, D], F32, tag="mm", name="mm")
                for js in range(jt + 1):
                    nc.tensor.matmul(
                        ps_ykv,
                        lhsT=sp_tiles[js][:, jt * P:(jt + 1) * P],
                        rhs=x16_tiles[js],
                        start=(js == 0), stop=(js == jt),
                    )
                ykv_t = ykvp.tile([P, D], F16, tag="ykv", name="ykv")
                _ln_apply(nc, stp, ykv_t, ps_ykv, eps_sb)
                for d in range(DT):
                    ps_tr = ps_mm.tile([P, P], F16, tag="mm", name="mmt")
                    nc.tensor.transpose(ps_tr, ykv_t[:, d * P:(d + 1) * P], ident_sb)
                    nc.vector.tensor_copy(
                        out=ykvT_tiles[d][:, jt * P:(jt + 1) * P], in_=ps_tr)

            # ---- phase C: y_sparse, gate, decoder GEMM partial ----
            ymlp_ps = [ps_acc.tile([P, D], F32, tag="acc", name="acc")
                       for _ in range(TP)]
            for k in range(NT):
                dec_k = decp.tile([P, D], F16, tag="dec", name="dec")
                nc.sync.dma_start(out=dec_k, in_=dec_r[k])
                ps_y = ps_mm.tile([P, T], F32, tag="mm", name="mm")
                for d in range(DT):
                    nc.tensor.matmul(
                        ps_y,
                        lhsT=encv_sb[:, d, k * P:(k + 1) * P],
                        rhs=ykvT_tiles[d],
                        start=(d == 0), stop=(d == DT - 1),
                    )
                # xy = relu(ps_y) * xs_k  (fp16 for the decoder GEMM)
                ys_k = ysp.tile([P, T], F16, tag="ys", name="ys")
                nc.scalar.activation(out=ys_k, in_=ps_y, func=Act.Relu)
                xy_k = xyp.tile([P, T], F16, tag="xy", name="xy")
                nc.vector.tensor_tensor(out=xy_k, in0=ys_k, in1=xs_tiles[k],
                                        op=Alu.mult)
                if last_layer:
                    nc.vector.reduce_sum(out=trace_sb[:, k:k + 1], in_=xy_k,
                                         axis=mybir.AxisListType.X)
                for jt in range(TP):
                    nc.tensor.matmul(
                        ymlp_ps[jt],
                        lhsT=xy_k[:, jt * P:(jt + 1) * P],
                        rhs=dec_k,
                        start=(k == 0), stop=(k == NT - 1),
                    )

            # ---- phase D: AllReduce ymlp over the 4-core head group ----
            for jt in range(TP):
                ym_sb = wp.tile([P, D], F16, tag="w16", name="w16")
                nc.scalar.activation(out=ym_sb, in_=ymlp_ps[jt], func=Act.Copy)
                nc.sync.dma_start(out=ar_in_r[jt], in_=ym_sb)
            if os.environ.get("BDH_SKIP_AR") == "1":
                # timing-only variant: wrong math, same data volume
                nc.sync.dma_start(out=ar_out.ap(), in_=ar_in.ap())
            else:
                nc.gpsimd.collective_compute(
                    "AllReduce", Alu.add,
                    replica_groups=REPLICA_GROUPS,
                    ins=[ar_in.ap()],
                    outs=[ar_out.ap()],
                )

            # x = LN(x + LN(ymlp)); also refresh fp16 shadow + x^T
            new_x, new_x16 = [], []
            for jt in range(TP):
                ym_t = wp.tile([P, D], F16, tag="w16", name="w16")
                nc.sync.dma_start(out=ym_t, in_=ar_out_r[jt])
                ln1 = wp.tile([P, D], F32, tag="w", name="w")
                _ln_apply(nc, stp, ln1, ym_t, eps_sb)
                nc.vector.tensor_add(ln1, ln1, x_tiles[jt])
                x_new = xp.tile([P, D], F32, tag="x", name="x")
                mu, rs = _ln_stats(nc, stp, ln1, eps_sb)
                nc.vector.tensor_scalar(
                    out=x_new, in0=ln1, scalar1=mu, scalar2=rs,
                    op0=Alu.subtract, op1=Alu.mult)
                x16_new = x16p.tile([P, D], F16, tag="x16", name="x16")
                nc.vector.tensor_scalar(
                    out=x16_new, in0=ln1, scalar1=mu, scalar2=rs,
                    op0=Alu.subtract, op1=Alu.mult)
                new_x.append(x_new)
                new_x16.append(x16_new)
            x_tiles, x16_tiles = new_x, new_x16
            new_xT = [xTp.tile([P, T], F16, tag="xT", name="xT")
                      for _ in range(DT)]
            for d in range(DT):
                for jt in range(TP):
                    ps_tr = ps_mm.tile([P, P], F16, tag="mm", name="mmt")
                    nc.tensor.transpose(
                        ps_tr, x16_tiles[jt][:, d * P:(d + 1) * P], ident_sb)
                    nc.vector.tensor_copy(
                        out=new_xT[d][:, jt * P:(jt + 1) * P], in_=ps_tr)
            xT_tiles = new_xT

        # -------- epilogue: logits, embedding, neuron trace --------
        for jt in range(TP):
            ps_lg = ps_mm.tile([P, V], F32, tag="mm", name="mm")
            for d in range(DT):
                nc.tensor.matmul(
                    ps_lg,
                    lhsT=xT_tiles[d][:, jt * P:(jt + 1) * P],
                    rhs=lm_sb[:, d, :],
                    start=(d == 0), stop=(d == DT - 1),
                )
            lg_sb = wp.tile([P, V], F32, tag="w", name="w")
            nc.scalar.activation(out=lg_sb, in_=ps_lg, func=Act.Copy)
            nc.sync.dma_start(out=logits_r[jt], in_=lg_sb)

        ps_e = ps_mm.tile([1, D], F32, tag="mm", name="emb")
        for jt in range(TP):
            nc.tensor.matmul(
                ps_e, lhsT=ones_sb, rhs=x16_tiles[jt],
                start=(jt == 0), stop=(jt == TP - 1),
            )
        e_sb = wp.tile([1, D], F32, tag="embs", name="embs")
        nc.vector.tensor_scalar_mul(e_sb, ps_e, 1.0 / T)
        nc.sync.dma_start(out=emb_d.ap(), in_=e_sb)

        tr_sb = wp.tile([P, NT], F32, tag="trs", name="trs")
        nc.vector.tensor_scalar_mul(tr_sb, trace_sb, 1.0 / T)
        nc.sync.dma_start(out=trace_d.ap(), in_=tr_sb)

    # TRN2 allows only one sync-wait per compute instruction; these are the
    # Bacc-pipeline passes that move matmul waits onto ldweights and split
    # the remaining excess onto EventSemaphore instructions.  The
    # run_bass_kernel_spmd/axon compile path does not run them itself.
    import bass_rust
    bass_rust.move_matmul_waits_to_ldweights(nc.m)
    bass_rust.generate_event_semaphores(nc)

    return nc


# ---------------------------------------------------------------------------
# Host side
# ---------------------------------------------------------------------------

def _rope_tables():
    """(NPAIR, T) cos/sin tables, matching the reference's f32 arithmetic."""
    qidx = np.float32(2.0) * np.floor(
        np.arange(N, dtype=np.float32) / np.float32(2.0))
    freqs = (np.float32(1.0)
             / np.power(np.float32(THETA), qidx / np.float32(N))
             / np.float32(2.0 * math.pi)).astype(np.float32)
    phases = (np.arange(T, dtype=np.float32)[:, None] * freqs[None, :])
    ph = np.mod(phases, np.float32(1.0)) * np.float32(2.0 * math.pi)
    cos_full = np.cos(ph).astype(np.float32)   # (T, N)
    sin_full = np.sin(ph).astype(np.float32)
    cosP = np.ascontiguousarray(cos_full[:, 0::2].T)   # (NPAIR, T)
    sinP = np.ascontiguousarray(sin_full[:, 0::2].T)
    return cosP, sinP


def prepare_in_maps(input_ids, embed_table, encoder, encoder_v, decoder, lm_head):
    input_ids = np.asarray(input_ids).astype(np.int64)
    embed_table = np.asarray(embed_table, dtype=np.float32)
    encoder = np.asarray(encoder, dtype=np.float32)
    encoder_v = np.asarray(encoder_v, dtype=np.float32)
    decoder = np.asarray(decoder, dtype=np.float32)
    lm_head = np.asarray(lm_head, dtype=np.float32)

    # initial x = LN(embed[ids])
    x = embed_table[input_ids]                      # (B, T, D) f32
    mu = x.mean(-1, keepdims=True, dtype=np.float32)
    var = ((x - mu) ** 2).mean(-1, keepdims=True, dtype=np.float32)
    x0 = ((x - mu) / np.sqrt(var + np.float32(LN_EPS))).astype(np.float32)

    # parity-major neuron permutation: n' = (n % 2) * NPAIR + n // 2
    perm = np.concatenate([np.arange(0, N, 2), np.arange(1, N, 2)])
    enc_p = encoder[:, :, perm]                     # (NH, D, N)
    encv_p = encoder_v[:, :, perm]
    dec_p = decoder.reshape(NH, N, D)[:, perm, :]   # head-major like reference

    cosP, sinP = _rope_tables()
    cosP = cosP.astype(NP_F16)
    sinP = sinP.astype(NP_F16)
    lm16 = np.ascontiguousarray(lm_head.astype(NP_F16))

    in_maps = []
    for c in range(N_CORES):
        b, h = divmod(c, NH)
        in_maps.append({
            "x0": np.ascontiguousarray(x0[b]),
            "x016": np.ascontiguousarray(x0[b].astype(NP_F16)),
            "xT0": np.ascontiguousarray(x0[b].T.astype(NP_F16)),
            "enc": np.ascontiguousarray(enc_p[h].astype(NP_F16)),
            "encv": np.ascontiguousarray(encv_p[h].astype(NP_F16)),
            "dec": np.ascontiguousarray(dec_p[h].astype(NP_F16)),
            "cosT": cosP,
            "sinT": sinP,
            "lm_head": lm16,
        })
    return in_maps


def assemble_outputs(results):
    """results: list of 8 dicts with 'logits', 'emb', 'trace'."""
    logits = np.stack([np.asarray(results[b * NH]["logits"], dtype=np.float32)
                       for b in range(B)])                     # (B, T, V)
    embedding = np.stack([np.asarray(results[b * NH]["emb"], dtype=np.float32)[0]
                          for b in range(B)])                  # (B, D)
    # device trace[p, k] = mean_t xy at permuted neuron n' = k*128 + p
    nprime = (np.arange(N) % 2) * NPAIR + np.arange(N) // 2
    neuron_trace = np.zeros((B, NH * N), dtype=np.float32)
    for c in range(N_CORES):
        b, h = divmod(c, NH)
        t_dev = np.asarray(results[c]["trace"], dtype=np.float32)  # (P, NT)
        flat = t_dev.T.reshape(-1)                                 # index n'
        neuron_trace[b, h * N:(h + 1) * N] = flat[nprime]
    return logits, embedding, neuron_trace


_NC_CACHE = {}


def _get_nc():
    if "nc" not in _NC_CACHE:
        _NC_CACHE["nc"] = build_nc()
    return _NC_CACHE["nc"]


def run_on_hw(in_maps, trace=False):
    from concourse.bass_utils import run_bass_kernel_spmd
    nc = _get_nc()
    return run_bass_kernel_spmd(nc, in_maps, list(range(N_CORES)), trace=trace)


def kernel(input_ids, embed_table, encoder, encoder_v, decoder, lm_head):
    in_maps = prepare_in_maps(
        input_ids, embed_table, encoder, encoder_v, decoder, lm_head)
    res = run_on_hw(in_maps)
    return assemble_outputs(res.results)


# revision 14
# speedup vs baseline: 2.3090x; 1.3800x over previous
"""Trainium2 Bass kernel for nn_BDHEncoder (BDH encoder, 6 tied layers).

Sharding: 8 cores = data-parallel over batch (2) x tensor-parallel over heads (4).
Each core computes its (batch, head) slice of every layer; the decoder GEMM
partial sums are AllReduced within each 4-core batch group; LayerNorm + residual
are computed replicated on every core of the group.

Device layout notes:
  - Neuron axis is permuted parity-major on the host (even originals first,
    odd second) so that RoPE pairs (2k, 2k+1) live at rows k and k+N/2 --
    tile i pairs with tile i+16 and shares one cos/sin (2048, T) table row k.
  - Activations x_sparse / qr / y_sparse are kept transposed: (N on partitions,
    T on free axis).  scores^T is built directly in (s, t) layout so it can be
    the stationary matmul operand of the ykv GEMM without a transpose.
  - Matmul operands are float16 (full PE rate, ~2^-11 rounding); PSUM
    accumulation, the x residual stream, RoPE tables/products and all LN
    statistics stay float32.
"""

import math
import os
import sys
from contextlib import ExitStack

import numpy as np

for _p in ("/opt/trn_rl_repo",):
    if os.path.isdir(_p) and _p not in sys.path:
        sys.path.insert(0, _p)

import concourse.bass as bass
import concourse.tile as tile
from concourse import mybir
from concourse.masks import make_identity, make_upper_triangular

# Problem constants (hardcoded per the self-contained-kernel contract).
B, T, D, NH, V = 2, 512, 256, 4, 256
N = 4096
NPAIR = N // 2
N_LAYER = 6
THETA = 2.0 ** 16
LN_EPS = 1e-5

P = 128
NT = N // P      # 32 neuron tiles
TP = T // P      # 4 t tiles
DT = D // P      # 2 d tiles
NPT = NPAIR // P # 16 pair tiles

F32 = mybir.dt.float32
F16 = mybir.dt.float16
NP_F16 = np.float16

N_CORES = 8
REPLICA_GROUPS = [[0, 1, 2, 3], [4, 5, 6, 7]]

# scores^T tile j covers free (t) range [SC_OFF[j], SC_OFF[j] + SC_W[j]).
SC_OFF = (0, 128, 256, 384)
SC_W = (512, 384, 256, 128)

Alu = mybir.AluOpType
Act = mybir.ActivationFunctionType


def _ln_stats(nc, stp, in_ap, eps_sb):
    """Return (mean, rstd) APs for LayerNorm along the free axis."""
    st = stp.tile([P, nc.vector.BN_STATS_DIM], F32, tag="bn", name="bn")
    nc.vector.bn_stats(out=st, in_=in_ap)
    mv = stp.tile([P, nc.vector.BN_AGGR_DIM], F32, tag="mv", name="mv")
    nc.vector.bn_aggr(out=mv, in_=st)
    rs = stp.tile([P, 1], F32, tag="rs", name="rs")
    nc.scalar.activation(out=rs, in_=mv[:, 1:2], func=Act.Sqrt, bias=eps_sb, scale=1.0)
    nc.vector.reciprocal(out=rs, in_=rs)
    return mv[:, 0:1], rs


def _ln_apply(nc, stp, out_ap, in_ap, eps_sb):
    mu, rs = _ln_stats(nc, stp, in_ap, eps_sb)
    nc.vector.tensor_scalar(
        out=out_ap, in0=in_ap, scalar1=mu, scalar2=rs,
        op0=Alu.subtract, op1=Alu.mult,
    )


def build_nc():
    nc = bass.Bass()

    # -------- I/O (fp16 for matmul operands, f32 elsewhere) --------
    x0_d = nc.declare_dram_parameter("x0", [T, D], F32, isOutput=False)
    xT0_d = nc.declare_dram_parameter("xT0", [D, T], F16, isOutput=False)
    x016_d = nc.declare_dram_parameter("x016", [T, D], F16, isOutput=False)
    enc_d = nc.declare_dram_parameter("enc", [D, N], F16, isOutput=False)
    encv_d = nc.declare_dram_parameter("encv", [D, N], F16, isOutput=False)
    dec_d = nc.declare_dram_parameter("dec", [N, D], F16, isOutput=False)
    cos_d = nc.declare_dram_parameter("cosT", [NPAIR, T], F16, isOutput=False)
    sin_d = nc.declare_dram_parameter("sinT", [NPAIR, T], F16, isOutput=False)
    lm_d = nc.declare_dram_parameter("lm_head", [D, V], F16, isOutput=False)

    logits_d = nc.declare_dram_parameter("logits", [T, V], F32, isOutput=True)
    emb_d = nc.declare_dram_parameter("emb", [1, D], F32, isOutput=True)
    trace_d = nc.declare_dram_parameter("trace", [P, NT], F32, isOutput=True)

    # Collective bounce buffers (collectives cannot touch I/O tensors).
    ar_in = nc.dram_tensor("ar_in", [T, D], F16)
    ar_out = nc.dram_tensor("ar_out", [T, D], F16)

    enc_r = enc_d.ap().rearrange("(dt p) n -> p dt n", p=P)
    encv_r = encv_d.ap().rearrange("(dt p) n -> p dt n", p=P)
    lm_r = lm_d.ap().rearrange("(dt p) v -> p dt v", p=P)
    x0_r = x0_d.ap().rearrange("(j p) d -> j p d", p=P)
    x016_r = x016_d.ap().rearrange("(j p) d -> j p d", p=P)
    xT0_r = xT0_d.ap().rearrange("(d p) t -> d p t", p=P)
    cos_r = cos_d.ap().rearrange("(i p) t -> i p t", p=P)
    sin_r = sin_d.ap().rearrange("(i p) t -> i p t", p=P)
    dec_r = dec_d.ap().rearrange("(k p) d -> k p d", p=P)
    ar_in_r = ar_in.ap().rearrange("(j p) d -> j p d", p=P)
    ar_out_r = ar_out.ap().rearrange("(j p) d -> j p d", p=P)
    logits_r = logits_d.ap().rearrange("(j p) v -> j p v", p=P)

    with ExitStack() as ctx:
        tc = ctx.enter_context(tile.TileContext(nc))
        constp = ctx.enter_context(tc.tile_pool(name="const", bufs=1))
        xsp = ctx.enter_context(tc.tile_pool(name="xs", bufs=NT))
        xyp = ctx.enter_context(tc.tile_pool(name="xy", bufs=NT))
        xp = ctx.enter_context(tc.tile_pool(name="x", bufs=2 * TP))
        x16p = ctx.enter_context(tc.tile_pool(name="x16", bufs=2 * TP))
        xTp = ctx.enter_context(tc.tile_pool(name="xT", bufs=2 * DT))
        spp = ctx.enter_context(tc.tile_pool(name="sp", bufs=TP))
        ykvp = ctx.enter_context(tc.tile_pool(name="ykv", bufs=2))
        ykvTp = ctx.enter_context(tc.tile_pool(name="ykvT", bufs=DT))
        wp = ctx.enter_context(tc.tile_pool(name="work", bufs=10))
        qp = ctx.enter_context(tc.tile_pool(name="q", bufs=6))
        csp = ctx.enter_context(tc.tile_pool(name="cs", bufs=6))
        ysp = ctx.enter_context(tc.tile_pool(name="ys", bufs=6))
        decp = ctx.enter_context(tc.tile_pool(name="decs", bufs=8))
        stp = ctx.enter_context(tc.tile_pool(name="stats", bufs=6))
        ps_acc = ctx.enter_context(tc.tile_pool(name="ps_acc", bufs=4, space="PSUM"))
        ps_mm = ctx.enter_context(tc.tile_pool(name="ps_mm", bufs=3, space="PSUM"))

        # -------- persistent constants --------
        enc_sb = constp.tile([P, DT, N], F16, tag="enc")
        nc.sync.dma_start(out=enc_sb, in_=enc_r)
        encv_sb = constp.tile([P, DT, N], F16, tag="encv")
        nc.sync.dma_start(out=encv_sb, in_=encv_r)
        lm_sb = constp.tile([P, DT, V], F16, tag="lm")
        nc.sync.dma_start(out=lm_sb, in_=lm_r)
        mask_sb = constp.tile([P, P], F32, tag="mask")
        make_upper_triangular(nc, mask_sb, 1.0, diag=False)  # 1 where s < t
        ident_sb = constp.tile([P, P], F16, tag="ident")
        make_identity(nc, ident_sb)
        eps_sb = constp.tile([P, 1], F32, tag="eps")
        nc.vector.memset(eps_sb, LN_EPS)
        ones_sb = constp.tile([P, 1], F16, tag="ones")
        nc.vector.memset(ones_sb, 1.0)
        trace_sb = constp.tile([P, NT], F32, tag="trace")

        x_tiles = []     # f32 residual stream
        x16_tiles = []   # fp16 shadow for matmul rhs
        for jt in range(TP):
            xt = xp.tile([P, D], F32, tag="x", name="x")
            nc.sync.dma_start(out=xt, in_=x0_r[jt])
            x_tiles.append(xt)
            xt16 = x16p.tile([P, D], F16, tag="x16", name="x16")
            nc.sync.dma_start(out=xt16, in_=x016_r[jt])
            x16_tiles.append(xt16)
        xT_tiles = []
        for d in range(DT):
            xtt = xTp.tile([P, T], F16, tag="xT", name="xT")
            nc.sync.dma_start(out=xtt, in_=xT0_r[d])
            xT_tiles.append(xtt)

        for layer in range(N_LAYER):
            last_layer = layer == N_LAYER - 1

            # ---- phase A: x_sparse (transposed) + RoPE + scores^T ----
            sc_ps = [ps_acc.tile([P, 512], F32, tag="acc", name="acc")[:, : SC_W[j]]
                     for j in range(TP)]
            xs_tiles = [None] * NT
            for i in range(NPT):
                for k in (i, NPT + i):
                    ps_x = ps_mm.tile([P, T], F32, tag="mm", name="mm")
                    for d in range(DT):
                        nc.tensor.matmul(
                            ps_x,
                            lhsT=enc_sb[:, d, k * P:(k + 1) * P],
                            rhs=xT_tiles[d],
                            start=(d == 0), stop=(d == DT - 1),
                        )
                    xs_k = xsp.tile([P, T], F16, tag="xs", name="xs")
                    nc.scalar.activation(out=xs_k, in_=ps_x, func=Act.Relu)
                    xs_tiles[k] = xs_k

                cos_i = csp.tile([P, T], F16, tag="cos", name="cos")
                nc.sync.dma_start(out=cos_i, in_=cos_r[i])
                sin_i = csp.tile([P, T], F16, tag="sin", name="sin")
                nc.sync.dma_start(out=sin_i, in_=sin_r[i])

                xe, xo = xs_tiles[i], xs_tiles[NPT + i]
                t1 = wp.tile([P, T], F16, tag="w", name="w")
                nc.vector.tensor_tensor(out=t1, in0=xe, in1=cos_i, op=Alu.mult)
                t2 = wp.tile([P, T], F16, tag="w", name="w")
                nc.vector.tensor_tensor(out=t2, in0=xo, in1=sin_i, op=Alu.mult)
                qe = qp.tile([P, T], F16, tag="q", name="q")
                nc.vector.tensor_sub(qe, t1, t2)
                t3 = wp.tile([P, T], F16, tag="w", name="w")
                nc.vector.tensor_tensor(out=t3, in0=xo, in1=cos_i, op=Alu.mult)
                t4 = wp.tile([P, T], F16, tag="w", name="w")
                nc.vector.tensor_tensor(out=t4, in0=xe, in1=sin_i, op=Alu.mult)
                qo = qp.tile([P, T], F16, tag="q", name="q")
                nc.vector.tensor_add(qo, t3, t4)

                for qi, q in enumerate((qe, qo)):
                    first = (i == 0) and (qi == 0)
                    last = (i == NPT - 1) and (qi == 1)
                    for j in range(TP):
                        nc.tensor.matmul(
                            sc_ps[j],
                            lhsT=q[:, j * P:(j + 1) * P],
                            rhs=q[:, SC_OFF[j]:SC_OFF[j] + SC_W[j]],
                            start=first, stop=last,
                        )

            # ---- phase B: mask scores, ykv = LN(scores @ x), transpose ----
            sp_tiles = []
            for j in range(TP):
                spt = spp.tile([P, T], F16, tag="sp", name="sp")
                # diagonal block: strictly-causal mask (s < t)
                nc.vector.tensor_tensor(
                    out=spt[:, j * P:(j + 1) * P],
                    in0=sc_ps[j][:, j * P - SC_OFF[j]:(j + 1) * P - SC_OFF[j]],
                    in1=mask_sb, op=Alu.mult,
                )
                if (j + 1) * P < T:
                    nc.scalar.activation(
                        out=spt[:, (j + 1) * P:],
                        in_=sc_ps[j][:, (j + 1) * P - SC_OFF[j]:],
                        func=Act.Copy,
                    )
                sp_tiles.append(spt)

            ykvT_tiles = [ykvTp.tile([P, T], F16, tag="ykvT", name="ykvT")
                          for _ in range(DT)]
            for jt in range(TP):
                ps_ykv = ps_mm.tile([P, D], F32, tag="mm", name="mm")
                for js in range(jt + 1):
                    nc.tensor.matmul(
                        ps_ykv,
                        lhsT=sp_tiles[js][:, jt * P:(jt + 1) * P],
                        rhs=x16_tiles[js],
                        start=(js == 0), stop=(js == jt),
                    )
                ykv_t = ykvp.tile([P, D], F16, tag="ykv", name="ykv")
                _ln_apply(nc, stp, ykv_t, ps_ykv, eps_sb)
                for d in range(DT):
                    ps_tr = ps_mm.tile([P, P], F16, tag="mm", name="mmt")
                    nc.tensor.transpose(ps_tr, ykv_t[:, d * P:(d + 1) * P], ident_sb)
                    nc.vector.tensor_copy(
                        out=ykvT_tiles[d][:, jt * P:(jt + 1) * P], in_=ps_tr)

            # ---- phase C: y_sparse, gate, decoder GEMM partial ----
            ymlp_ps = [ps_acc.tile([P, D], F32, tag="acc", name="acc")
                       for _ in range(TP)]
            for k in range(NT):
                dec_k = decp.tile([P, D], F16, tag="dec", name="dec")
                nc.sync.dma_start(out=dec_k, in_=dec_r[k])
                ps_y = ps_mm.tile([P, T], F32, tag="mm", name="mm")
                for d in range(DT):
                    nc.tensor.matmul(
                        ps_y,
                        lhsT=encv_sb[:, d, k * P:(k + 1) * P],
                        rhs=ykvT_tiles[d],
                        start=(d == 0), stop=(d == DT - 1),
                    )
                # xy = relu(ps_y) * xs_k  (fp16 for the decoder GEMM)
                ys_k = ysp.tile([P, T], F16, tag="ys", name="ys")
                nc.scalar.activation(out=ys_k, in_=ps_y, func=Act.Relu)
                xy_k = xyp.tile([P, T], F16, tag="xy", name="xy")
                nc.vector.tensor_tensor(out=xy_k, in0=ys_k, in1=xs_tiles[k],
                                        op=Alu.mult)
                if last_layer:
                    nc.vector.reduce_sum(out=trace_sb[:, k:k + 1], in_=xy_k,
                                         axis=mybir.AxisListType.X)
                for jt in range(TP):
                    nc.tensor.matmul(
                        ymlp_ps[jt],
                        lhsT=xy_k[:, jt * P:(jt + 1) * P],
                        rhs=dec_k,
                        start=(k == 0), stop=(k == NT - 1),
                    )

            # ---- phase D: AllReduce ymlp over the 4-core head group ----
            for jt in range(TP):
                ym_sb = wp.tile([P, D], F16, tag="w16", name="w16")
                nc.scalar.activation(out=ym_sb, in_=ymlp_ps[jt], func=Act.Copy)
                nc.sync.dma_start(out=ar_in_r[jt], in_=ym_sb)
            if os.environ.get("BDH_SKIP_AR") == "1":
                # timing-only variant: wrong math, same data volume
                nc.sync.dma_start(out=ar_out.ap(), in_=ar_in.ap())
            else:
                nc.gpsimd.collective_compute(
                    "AllReduce", Alu.add,
                    replica_groups=REPLICA_GROUPS,
                    ins=[ar_in.ap()],
                    outs=[ar_out.ap()],
                )

            # x = LN(x + LN(ymlp)); also refresh fp16 shadow + x^T
            new_x, new_x16 = [], []
            for jt in range(TP):
                ym_t = wp.tile([P, D], F16, tag="w16", name="w16")
                nc.sync.dma_start(out=ym_t, in_=ar_out_r[jt])
                ln1 = wp.tile([P, D], F32, tag="w", name="w")
                _ln_apply(nc, stp, ln1, ym_t, eps_sb)
                nc.vector.tensor_add(ln1, ln1, x_tiles[jt])
                x_new = xp.tile([P, D], F32, tag="x", name="x")
                mu, rs = _ln_stats(nc, stp, ln1, eps_sb)
                nc.vector.tensor_scalar(
                    out=x_new, in0=ln1, scalar1=mu, scalar2=rs,
                    op0=Alu.subtract, op1=Alu.mult)
                x16_new = x16p.tile([P, D], F16, tag="x16", name="x16")
                nc.vector.tensor_scalar(
                    out=x16_new, in0=ln1, scalar1=mu, scalar2=rs,
                    op0=Alu.subtract, op1=Alu.mult)
                new_x.append(x_new)
                new_x16.append(x16_new)
            x_tiles, x16_tiles = new_x, new_x16
            new_xT = [xTp.tile([P, T], F16, tag="xT", name="xT")
                      for _ in range(DT)]
            for d in range(DT):
                for jt in range(TP):
                    ps_tr = ps_mm.tile([P, P], F16, tag="mm", name="mmt")
                    nc.tensor.transpose(
                        ps_tr, x16_tiles[jt][:, d * P:(d + 1) * P], ident_sb)
                    nc.vector.tensor_copy(
                        out=new_xT[d][:, jt * P:(jt + 1) * P], in_=ps_tr)
            xT_tiles = new_xT

        # -------- epilogue: logits, embedding, neuron trace --------
        for jt in range(TP):
            ps_lg = ps_mm.tile([P, V], F32, tag="mm", name="mm")
            for d in range(DT):
                nc.tensor.matmul(
                    ps_lg,
                    lhsT=xT_tiles[d][:, jt * P:(jt + 1) * P],
                    rhs=lm_sb[:, d, :],
                    start=(d == 0), stop=(d == DT - 1),
                )
            lg_sb = wp.tile([P, V], F32, tag="w", name="w")
            nc.scalar.activation(out=lg_sb, in_=ps_lg, func=Act.Copy)
            nc.sync.dma_start(out=logits_r[jt], in_=lg_sb)

        ps_e = ps_mm.tile([1, D], F32, tag="mm", name="emb")
        for jt in range(TP):
            nc.tensor.matmul(
                ps_e, lhsT=ones_sb, rhs=x16_tiles[jt],
                start=(jt == 0), stop=(jt == TP - 1),
            )
        e_sb = wp.tile([1, D], F32, tag="embs", name="embs")
        nc.vector.tensor_scalar_mul(e_sb, ps_e, 1.0 / T)
        nc.sync.dma_start(out=emb_d.ap(), in_=e_sb)

        tr_sb = wp.tile([P, NT], F32, tag="trs", name="trs")
        nc.vector.tensor_scalar_mul(tr_sb, trace_sb, 1.0 / T)
        nc.sync.dma_start(out=trace_d.ap(), in_=tr_sb)

    # TRN2 allows only one sync-wait per compute instruction; these are the
    # Bacc-pipeline passes that move matmul waits onto ldweights and split
    # the remaining excess onto EventSemaphore instructions.  The
    # run_bass_kernel_spmd/axon compile path does not run them itself.
    import bass_rust
    bass_rust.move_matmul_waits_to_ldweights(nc.m)
    bass_rust.generate_event_semaphores(nc)

    return nc


# ---------------------------------------------------------------------------
# Host side
# ---------------------------------------------------------------------------

def _rope_tables():
    """(NPAIR, T) cos/sin tables, matching the reference's f32 arithmetic."""
    qidx = np.float32(2.0) * np.floor(
        np.arange(N, dtype=np.float32) / np.float32(2.0))
    freqs = (np.float32(1.0)
             / np.power(np.float32(THETA), qidx / np.float32(N))
             / np.float32(2.0 * math.pi)).astype(np.float32)
    phases = (np.arange(T, dtype=np.float32)[:, None] * freqs[None, :])
    ph = np.mod(phases, np.float32(1.0)) * np.float32(2.0 * math.pi)
    cos_full = np.cos(ph).astype(np.float32)   # (T, N)
    sin_full = np.sin(ph).astype(np.float32)
    cosP = np.ascontiguousarray(cos_full[:, 0::2].T)   # (NPAIR, T)
    sinP = np.ascontiguousarray(sin_full[:, 0::2].T)
    return cosP, sinP


def prepare_in_maps(input_ids, embed_table, encoder, encoder_v, decoder, lm_head):
    input_ids = np.asarray(input_ids).astype(np.int64)
    embed_table = np.asarray(embed_table, dtype=np.float32)
    encoder = np.asarray(encoder, dtype=np.float32)
    encoder_v = np.asarray(encoder_v, dtype=np.float32)
    decoder = np.asarray(decoder, dtype=np.float32)
    lm_head = np.asarray(lm_head, dtype=np.float32)

    # initial x = LN(embed[ids])
    x = embed_table[input_ids]                      # (B, T, D) f32
    mu = x.mean(-1, keepdims=True, dtype=np.float32)
    var = ((x - mu) ** 2).mean(-1, keepdims=True, dtype=np.float32)
    x0 = ((x - mu) / np.sqrt(var + np.float32(LN_EPS))).astype(np.float32)

    # parity-major neuron permutation: n' = (n % 2) * NPAIR + n // 2
    perm = np.concatenate([np.arange(0, N, 2), np.arange(1, N, 2)])
    enc_p = encoder[:, :, perm]                     # (NH, D, N)
    encv_p = encoder_v[:, :, perm]
    dec_p = decoder.reshape(NH, N, D)[:, perm, :]   # head-major like reference

    cosP, sinP = _rope_tables()
    cosP = cosP.astype(NP_F16)
    sinP = sinP.astype(NP_F16)
    lm16 = np.ascontiguousarray(lm_head.astype(NP_F16))

    in_maps = []
    for c in range(N_CORES):
        b, h = divmod(c, NH)
        in_maps.append({
            "x0": np.ascontiguousarray(x0[b]),
            "x016": np.ascontiguousarray(x0[b].astype(NP_F16)),
            "xT0": np.ascontiguousarray(x0[b].T.astype(NP_F16)),
            "enc": np.ascontiguousarray(enc_p[h].astype(NP_F16)),
            "encv": np.ascontiguousarray(encv_p[h].astype(NP_F16)),
            "dec": np.ascontiguousarray(dec_p[h].astype(NP_F16)),
            "cosT": cosP,
            "sinT": sinP,
            "lm_head": lm16,
        })
    return in_maps


def assemble_outputs(results):
    """results: list of 8 dicts with 'logits', 'emb', 'trace'."""
    logits = np.stack([np.asarray(results[b * NH]["logits"], dtype=np.float32)
                       for b in range(B)])                     # (B, T, V)
    embedding = np.stack([np.asarray(results[b * NH]["emb"], dtype=np.float32)[0]
                          for b in range(B)])                  # (B, D)
    # device trace[p, k] = mean_t xy at permuted neuron n' = k*128 + p
    nprime = (np.arange(N) % 2) * NPAIR + np.arange(N) // 2
    neuron_trace = np.zeros((B, NH * N), dtype=np.float32)
    for c in range(N_CORES):
        b, h = divmod(c, NH)
        t_dev = np.asarray(results[c]["trace"], dtype=np.float32)  # (P, NT)
        flat = t_dev.T.reshape(-1)                                 # index n'
        neuron_trace[b, h * N:(h + 1) * N] = flat[nprime]
    return logits, embedding, neuron_trace


_NC_CACHE = {}


def _get_nc():
    if "nc" not in _NC_CACHE:
        _NC_CACHE["nc"] = build_nc()
    return _NC_CACHE["nc"]


def run_on_hw(in_maps, trace=False):
    from concourse.bass_utils import run_bass_kernel_spmd
    nc = _get_nc()
    return run_bass_kernel_spmd(nc, in_maps, list(range(N_CORES)), trace=trace)


def kernel(input_ids, embed_table, encoder, encoder_v, decoder, lm_head):
    in_maps = prepare_in_maps(
        input_ids, embed_table, encoder, encoder_v, decoder, lm_head)
    res = run_on_hw(in_maps)
    return assemble_outputs(res.results)
